# revision 11
# baseline (speedup 1.0000x reference)
"""Multi-head causal attention (B=4, T=2048, D=1024, 16 heads) on 8 TRN2 cores.

Sharding: core c handles batch b = c//2 and head-group g = c%2 (8 of the 16
heads, i.e. 512 of the 1024 qkv feature dims).  Each core computes its head
group's QKV projections, causal attention, and a partial output projection
(columns of Wo belonging to its heads).  The host sums the two partials per
batch and adds the bias.

Per-core kernel layout (all matmul operands stored as float32r — full-rate
PE with ~1e-4 relative error):
  stage 1: qT[dh, t], kT[dh, t] (transposed) and v[t, dh] from xT j-tiles.
           v is stored with an interleaved ones-column per head ("vaug") so
           the PV matmul accumulates the softmax denominator for free.
  stage 2: per (tq-chunk 512, head): S^T blocks = kT_blk.T @ qT_chunk
           ([tk=128, tq=512] in PSUM), exp on ACT (no max subtraction --
           scores are O(1) so exp is safe in fp32), causal mask on the 4
           diagonal k-tiles via gpsimd affine_select, PV matmul accumulates
           outT[65, 512] (row 64 = denominator).  Normalize via DVE
           reciprocal + PE ones-broadcast + DVE multiply -> attnT[j, t].
  stage 3: partialT[i, t] = WoT_blk.T @ attnT chunks.
"""

import numpy as np

import concourse.bass as bass
import concourse.mybir as mybir
import concourse.tile as tile
from concourse.bass_utils import run_bass_kernel_spmd

B, T, D = 4, 2048, 1024
N_HEAD, HD = 16, 64
N_CORES = 8
GROUPS = 2            # head groups (cores per batch)
HPC = N_HEAD // GROUPS  # heads per core = 8
DG = HPC * HD           # feature dims per core = 512
NJT = D // 128          # 8 j-tiles over the model dim
NPR = DG // 128         # 4 dh-tiles (head pairs) per core
NTT = T // 128          # 16 t-tiles
NCH = T // 512          # 4 t-chunks
CH = 512

F32 = mybir.dt.float32
F32R = mybir.dt.float32r


def _split_excess_waits(nc, max_waits=1):
    """This walrus build encodes at most one sync-wait per instruction;
    Tile emits several.  Hoist surplus waits onto standalone same-engine
    NoOps placed immediately before the instruction."""
    for f in nc.m.functions:
        for bb in f.blocks:
            new = []
            for inst in bb.instructions:
                si = inst.sync_info
                waits = list(si.on_wait) if si is not None and si.on_wait else []
                if len(waits) > max_waits:
                    surplus, keep = waits[:-max_waits], waits[-max_waits:]
                    for k, w in enumerate(surplus):
                        nop = mybir.InstNoOp(name=f"{inst.name}-wsplit{k}", ins=[], outs=[])
                        nop.engine = inst.engine
                        nop.sync_info = mybir.SyncInfo(on_wait=[w], on_update=[])
                        new.append(nop)
                    inst.sync_info = mybir.SyncInfo(
                        on_wait=keep,
                        on_update=list(si.on_update) if si.on_update else [])
                new.append(inst)
            bb.instructions = new


def build_program():
    nc = bass.Bass("TRN2", target_bir_lowering=False, debug=False,
                   num_devices=N_CORES)

    xT = nc.dram_tensor("xT", [D, T], F32R, kind="ExternalInput")
    wqT = nc.dram_tensor("wqT", [D, DG], F32R, kind="ExternalInput")
    wkT = nc.dram_tensor("wkT", [D, DG], F32R, kind="ExternalInput")
    wvT = nc.dram_tensor("wvT", [D, DG], F32R, kind="ExternalInput")
    woT = nc.dram_tensor("woT", [DG, D], F32R, kind="ExternalInput")
    outT = nc.dram_tensor("outT", [D, T], F32, kind="ExternalOutput")

    with tile.TileContext(nc) as tc:
        _build_body(nc, tc, xT, wqT, wkT, wvT, woT, outT)
    _split_excess_waits(nc)
    return nc


def _build_body(nc, tc, xT, wqT, wkT, wvT, woT, outT):
    from contextlib import ExitStack
    est = ExitStack()
    with est:
        persist = est.enter_context(tc.tile_pool(name="persist", bufs=1))
        e_pool = est.enter_context(tc.tile_pool(name="epool", bufs=4))
        attn_pool = est.enter_context(tc.tile_pool(name="attnpool", bufs=2))
        ps_mm = est.enter_context(tc.tile_pool(name="ps_mm", bufs=2, space="PSUM"))
        ps_st = est.enter_context(tc.tile_pool(name="ps_st", bufs=2, space="PSUM"))
        ps_pv = est.enter_context(tc.tile_pool(name="ps_pv", bufs=1, space="PSUM"))

        # persistent tensors
        qT_sb = persist.tile([128, NPR, T], F32R)     # [dh%128, dh-tile, t]
        kT_sb = persist.tile([128, NPR, T], F32R)
        vaug = persist.tile([128, NTT, HPC, HD + 1], F32R)  # [t%128, t-tile, h, hd|1]
        # Memset can't write f32r on this toolchain; stage fp32 ones and
        # copy (the DVE copy performs the f32r rounding).
        ones_f32 = persist.tile([128, HD], F32)
        nc.vector.memset(ones_f32[:], 1.0)
        ones_bc = persist.tile([1, HD], F32R)
        nc.vector.tensor_copy(ones_bc[:], ones_f32[0:1, :])
        for tt in range(NTT):
            nc.vector.tensor_copy(vaug[:, tt, :, HD], ones_f32[:, 0:HPC])

        # ---- stage 1: projections (x streamed in T-halves) ----
        with tc.tile_pool(name="wqkv", bufs=1) as wpool, \
             tc.tile_pool(name="xhpool", bufs=1) as xpool:
            wq_sb = wpool.tile([128, NJT, DG], F32R)
            wk_sb = wpool.tile([128, NJT, DG], F32R)
            wv_sb = wpool.tile([128, NJT, DG], F32R)
            for jt in range(NJT):
                nc.sync.dma_start(out=wq_sb[:, jt, :], in_=wqT[128 * jt:128 * (jt + 1), :])
                nc.sync.dma_start(out=wk_sb[:, jt, :], in_=wkT[128 * jt:128 * (jt + 1), :])
                nc.sync.dma_start(out=wv_sb[:, jt, :], in_=wvT[128 * jt:128 * (jt + 1), :])

            for half in range(2):
                t0 = (T // 2) * half
                xh = xpool.tile([128, NJT, T // 2], F32R, tag="xh")
                for jt in range(NJT):
                    nc.sync.dma_start(out=xh[:, jt, :],
                                      in_=xT[128 * jt:128 * (jt + 1), t0:t0 + T // 2])
                # qT / kT: out[dh-tile, chunk] = sum_jt W[:, jt, dh-tile].T @ xh[:, jt, chunk]
                for wsb, dst in ((wq_sb, qT_sb), (wk_sb, kT_sb)):
                    for dt_ in range(NPR):
                        for chh in range(2):
                            ps = ps_mm.tile([128, CH], F32, tag="mm")
                            for jt in range(NJT):
                                nc.tensor.matmul(
                                    ps[:],
                                    lhsT=wsb[:, jt, 128 * dt_:128 * (dt_ + 1)],
                                    rhs=xh[:, jt, CH * chh:CH * (chh + 1)],
                                    start=(jt == 0), stop=(jt == NJT - 1))
                            nc.vector.tensor_copy(
                                dst[:, dt_, t0 + CH * chh:t0 + CH * (chh + 1)], ps[:])
                # v: out[t-tile, :] = sum_jt xh[:, jt, t-tile].T @ wv[:, jt, :]
                for tt in range(NTT // 2):
                    ps = ps_mm.tile([128, DG], F32, tag="mm")
                    for jt in range(NJT):
                        nc.tensor.matmul(
                            ps[:],
                            lhsT=xh[:, jt, 128 * tt:128 * (tt + 1)],
                            rhs=wv_sb[:, jt, :],
                            start=(jt == 0), stop=(jt == NJT - 1))
                    nc.vector.tensor_copy(
                        vaug[:, NTT // 2 * half + tt, :, 0:HD],
                        ps[:].rearrange("p (h d) -> p h d", h=HPC))

        # ---- stages 2+3 ----
        with tc.tile_pool(name="wopool", bufs=1) as wopool, \
             tc.tile_pool(name="outpool", bufs=2) as out_pool, \
             tc.tile_pool(name="bcpool", bufs=2) as bc_pool, \
             tc.tile_pool(name="smpool", bufs=4) as sm_pool:
            wo_sb = wopool.tile([128, NPR, D], F32R)
            for jt in range(NPR):
                nc.sync.dma_start(out=wo_sb[:, jt, :], in_=woT[128 * jt:128 * (jt + 1), :])

            for c in range(NCH):
                K = 4 * (c + 1)  # tk tiles needed for this tq chunk
                attn_sb = attn_pool.tile([128, NPR, CH], F32R, tag="attn")
                for hg in range(2):
                    heads = [4 * hg + i for i in range(4)]
                    pv = {}
                    for h in heads:
                        pv[h] = ps_pv.tile([HD + 1, CH], F32,
                                           tag=f"pv{h % 4}", name=f"pv_{c}_{h}")
                    for k in range(K):
                        for pr in (2 * hg, 2 * hg + 1):
                            for sub in range(2):
                                h = 2 * pr + sub
                                st = ps_st.tile([128, CH], F32, tag="st")
                                nc.tensor.matmul(
                                    st[:],
                                    lhsT=kT_sb[64 * sub:64 * (sub + 1), pr,
                                               128 * k:128 * (k + 1)],
                                    rhs=qT_sb[64 * sub:64 * (sub + 1), pr,
                                              CH * c:CH * (c + 1)],
                                    start=True, stop=True)
                                e = e_pool.tile([128, CH], F32R, tag="e")
                                nc.scalar.activation(
                                    out=e[:], in_=st[:],
                                    func=mybir.ActivationFunctionType.Exp,
                                    scale=0.125)
                                d = k - 4 * c
                                if d >= 0:
                                    # causal: keep E[p, n] where n >= p + 128*d
                                    nc.gpsimd.affine_select(
                                        out=e[:], in_=e[:],
                                        compare_op=mybir.AluOpType.is_ge,
                                        fill=0.0,
                                        base=-128 * d,
                                        pattern=[[1, CH]],
                                        channel_multiplier=-1)
                                nc.tensor.matmul(
                                    pv[h][:],
                                    lhsT=vaug[:, k, h, :],
                                    rhs=e[:],
                                    start=(k == 0), stop=(k == K - 1))
                    for h in heads:
                        pr, sub = h // 2, h % 2
                        recip = sm_pool.tile([1, CH], F32R, tag="recip")
                        with nc.allow_low_precision(
                                reason="f32r recip feeds f32r matmul broadcast"):
                            nc.vector.reciprocal(recip[:], pv[h][HD:HD + 1, :])
                        bc_ps = ps_st.tile([HD, CH], F32, tag="st")
                        nc.tensor.matmul(bc_ps[:], lhsT=ones_bc[:],
                                         rhs=recip[:], start=True, stop=True)
                        bc = bc_pool.tile([HD, CH], F32, tag="bc")
                        nc.vector.tensor_copy(bc[:], bc_ps[:])
                        nc.vector.tensor_mul(
                            attn_sb[64 * sub:64 * (sub + 1), pr, :],
                            pv[h][0:HD, :], bc[:])
                # output projection for this chunk
                for it in range(D // 128):
                    ps = ps_mm.tile([128, CH], F32, tag="mm")
                    for jt in range(NPR):
                        nc.tensor.matmul(
                            ps[:],
                            lhsT=wo_sb[:, jt, 128 * it:128 * (it + 1)],
                            rhs=attn_sb[:, jt, :],
                            start=(jt == 0), stop=(jt == NPR - 1))
                    stg = out_pool.tile([128, CH], F32, tag="stg")
                    nc.vector.tensor_copy(stg[:], ps[:])
                    nc.sync.dma_start(
                        out=outT[128 * it:128 * (it + 1), CH * c:CH * (c + 1)],
                        in_=stg[:])


_PROGRAM = None


def _get_program():
    global _PROGRAM
    if _PROGRAM is None:
        _PROGRAM = build_program()
    return _PROGRAM


def kernel(x, Wq, Wk, Wv, Wo, bo):
    x = np.asarray(x, np.float32)
    Wq = np.asarray(Wq, np.float32)
    Wk = np.asarray(Wk, np.float32)
    Wv = np.asarray(Wv, np.float32)
    Wo = np.asarray(Wo, np.float32)
    bo = np.asarray(bo, np.float32)

    nc = _get_program()
    in_maps = []
    for c in range(N_CORES):
        b, g = c // GROUPS, c % GROUPS
        rows = slice(DG * g, DG * (g + 1))
        in_maps.append({
            "xT": np.ascontiguousarray(x[b].T),
            "wqT": np.ascontiguousarray(Wq[rows, :].T),
            "wkT": np.ascontiguousarray(Wk[rows, :].T),
            "wvT": np.ascontiguousarray(Wv[rows, :].T),
            "woT": np.ascontiguousarray(Wo[:, rows].T),
        })
    res = run_bass_kernel_spmd(nc, in_maps, core_ids=list(range(N_CORES)))
    out = np.empty((B, T, D), np.float32)
    for b in range(B):
        acc = res.results[GROUPS * b]["outT"].astype(np.float32)
        for g in range(1, GROUPS):
            acc = acc + res.results[GROUPS * b + g]["outT"]
        out[b] = acc.T + bo
    return out


# revision 13
# speedup vs baseline: 1.0080x; 1.0080x over previous
"""Multi-head causal attention (B=4, T=2048, D=1024, 16 heads) on 8 TRN2 cores.

Sharding: core c handles batch b = c//2 and head-group g = c%2 (8 of the 16
heads, i.e. 512 of the 1024 qkv feature dims).  Each core computes its head
group's QKV projections, causal attention, and a partial output projection
(columns of Wo belonging to its heads).  The host sums the two partials per
batch and adds the bias.

Per-core kernel layout (all matmul operands stored as float32r — full-rate
PE with ~1e-4 relative error):
  stage 1: qT[dh, t], kT[dh, t] (transposed) and v[t, dh] from xT j-tiles.
           v is stored with an interleaved ones-column per head ("vaug") so
           the PV matmul accumulates the softmax denominator for free.
  stage 2: per (tq-chunk 512, head): S^T blocks = kT_blk.T @ qT_chunk
           ([tk=128, tq=512] in PSUM), exp on ACT (no max subtraction --
           scores are O(1) so exp is safe in fp32), causal mask on the 4
           diagonal k-tiles via gpsimd affine_select, PV matmul accumulates
           outT[65, 512] (row 64 = denominator).  Normalize via DVE
           reciprocal + PE ones-broadcast + DVE multiply -> attnT[j, t].
  stage 3: partialT[i, t] = WoT_blk.T @ attnT chunks.
"""

import numpy as np

import concourse.bass as bass
import concourse.mybir as mybir
import concourse.tile as tile
from concourse.bass_utils import run_bass_kernel_spmd

B, T, D = 4, 2048, 1024
N_HEAD, HD = 16, 64
N_CORES = 8
GROUPS = 2            # head groups (cores per batch)
HPC = N_HEAD // GROUPS  # heads per core = 8
DG = HPC * HD           # feature dims per core = 512
NJT = D // 128          # 8 j-tiles over the model dim
NPR = DG // 128         # 4 dh-tiles (head pairs) per core
NTT = T // 128          # 16 t-tiles
NCH = T // 512          # 4 t-chunks
CH = 512

F32 = mybir.dt.float32
F32R = mybir.dt.float32r


def _split_excess_waits(nc, max_waits=1):
    """This walrus build encodes at most one sync-wait per instruction;
    Tile emits several.  Hoist surplus waits onto standalone same-engine
    NoOps placed immediately before the instruction."""
    for f in nc.m.functions:
        for bb in f.blocks:
            new = []
            for inst in bb.instructions:
                si = inst.sync_info
                waits = list(si.on_wait) if si is not None and si.on_wait else []
                if len(waits) > max_waits:
                    surplus, keep = waits[:-max_waits], waits[-max_waits:]
                    for k, w in enumerate(surplus):
                        nop = mybir.InstNoOp(name=f"{inst.name}-wsplit{k}", ins=[], outs=[])
                        nop.engine = inst.engine
                        nop.sync_info = mybir.SyncInfo(on_wait=[w], on_update=[])
                        new.append(nop)
                    inst.sync_info = mybir.SyncInfo(
                        on_wait=keep,
                        on_update=list(si.on_update) if si.on_update else [])
                new.append(inst)
            bb.instructions = new


def build_program():
    nc = bass.Bass("TRN2", target_bir_lowering=False, debug=False,
                   num_devices=N_CORES)

    xT = nc.dram_tensor("xT", [D, T], F32R, kind="ExternalInput")
    wqT = nc.dram_tensor("wqT", [D, DG], F32R, kind="ExternalInput")
    wkT = nc.dram_tensor("wkT", [D, DG], F32R, kind="ExternalInput")
    wvT = nc.dram_tensor("wvT", [D, DG], F32R, kind="ExternalInput")
    woT = nc.dram_tensor("woT", [DG, D], F32R, kind="ExternalInput")
    outT = nc.dram_tensor("outT", [D, T], F32, kind="ExternalOutput")

    with tile.TileContext(nc) as tc:
        _build_body(nc, tc, xT, wqT, wkT, wvT, woT, outT)
    _split_excess_waits(nc)
    return nc


def _build_body(nc, tc, xT, wqT, wkT, wvT, woT, outT):
    from contextlib import ExitStack
    est = ExitStack()
    with est:
        persist = est.enter_context(tc.tile_pool(name="persist", bufs=1))
        e_pool = est.enter_context(tc.tile_pool(name="epool", bufs=4))
        attn_pool = est.enter_context(tc.tile_pool(name="attnpool", bufs=2))
        qt_pool = est.enter_context(tc.tile_pool(name="qtpool", bufs=2))
        out_pool = est.enter_context(tc.tile_pool(name="outpool", bufs=2))
        bc_pool = est.enter_context(tc.tile_pool(name="bcpool", bufs=2))
        sm_pool = est.enter_context(tc.tile_pool(name="smpool", bufs=2))
        wpool = est.enter_context(tc.tile_pool(name="wqkv", bufs=1))
        wopool = est.enter_context(tc.tile_pool(name="wopool", bufs=1))
        xpool = est.enter_context(tc.tile_pool(name="xhpool", bufs=1))
        ps_mm = est.enter_context(tc.tile_pool(name="ps_mm", bufs=1, space="PSUM"))
        ps_st = est.enter_context(tc.tile_pool(name="ps_st", bufs=3, space="PSUM"))
        ps_pv = est.enter_context(tc.tile_pool(name="ps_pv", bufs=1, space="PSUM"))

        # persistent tensors
        kT_sb = persist.tile([128, NPR, T], F32R)     # [dh%128, dh-tile, t]
        vaug = persist.tile([128, NTT, HPC, HD + 1], F32R)  # [t%128, t-tile, h, hd|1]
        # Memset can't write f32r on this toolchain; stage fp32 ones and
        # copy (the DVE copy performs the f32r rounding).
        ones_f32 = persist.tile([128, HD], F32)
        nc.vector.memset(ones_f32[:], 1.0)
        ones_bc = persist.tile([1, HD], F32R)
        nc.vector.tensor_copy(ones_bc[:], ones_f32[0:1, :])
        for tt in range(NTT):
            nc.vector.tensor_copy(vaug[:, tt, :, HD], ones_f32[:, 0:HPC])

        wq_sb = wpool.tile([128, NJT, DG], F32R)
        wk_sb = wpool.tile([128, NJT, DG], F32R)
        wv_sb = wpool.tile([128, NJT, DG], F32R)
        wo_sb = wopool.tile([128, NPR, D], F32R)
        xh = {}

        def dma_xh(q):
            xh[q] = xpool.tile([128, NJT, CH], F32R, tag="xh", name=f"xh{q}")
            for jt in range(NJT):
                nc.sync.dma_start(out=xh[q][:, jt, :],
                                  in_=xT[128 * jt:128 * (jt + 1), CH * q:CH * (q + 1)])

        # First quarter of x and wq go first so the PE can start ASAP.
        dma_xh(0)
        for jt in range(NJT):
            nc.sync.dma_start(out=wq_sb[:, jt, :], in_=wqT[128 * jt:128 * (jt + 1), :])
        for jt in range(NJT):
            nc.sync.dma_start(out=wk_sb[:, jt, :], in_=wkT[128 * jt:128 * (jt + 1), :])
        for jt in range(NJT):
            nc.sync.dma_start(out=wv_sb[:, jt, :], in_=wvT[128 * jt:128 * (jt + 1), :])
        for jt in range(NPR):
            nc.sync.dma_start(out=wo_sb[:, jt, :], in_=woT[128 * jt:128 * (jt + 1), :])

        for q in range(NCH):
            # ---- stage 1 for t-quarter q ----
            if q > 0:
                dma_xh(q)
            qTc = qt_pool.tile([128, NPR, CH], F32R, tag="qT", name=f"qT{q}")
            for wsb, dst in ((wq_sb, qTc), (wk_sb, None)):
                for dt_ in range(NPR):
                    ps = ps_mm.tile([128, CH], F32, tag="mm")
                    for jt in range(NJT):
                        nc.tensor.matmul(
                            ps[:],
                            lhsT=wsb[:, jt, 128 * dt_:128 * (dt_ + 1)],
                            rhs=xh[q][:, jt, :],
                            start=(jt == 0), stop=(jt == NJT - 1))
                    if dst is not None:
                        nc.vector.tensor_copy(dst[:, dt_, :], ps[:])
                    else:
                        nc.vector.tensor_copy(
                            kT_sb[:, dt_, CH * q:CH * (q + 1)], ps[:])
            for tt in range(4):  # t-tiles of this quarter
                ps = ps_mm.tile([128, DG], F32, tag="mm")
                for jt in range(NJT):
                    nc.tensor.matmul(
                        ps[:],
                        lhsT=xh[q][:, jt, 128 * tt:128 * (tt + 1)],
                        rhs=wv_sb[:, jt, :],
                        start=(jt == 0), stop=(jt == NJT - 1))
                nc.vector.tensor_copy(
                    vaug[:, 4 * q + tt, :, 0:HD],
                    ps[:].rearrange("p (h d) -> p h d", h=HPC))

            # ---- attention for tq chunk q ----
            c = q
            K = 4 * (c + 1)  # tk tiles needed for this tq chunk
            attn_sb = attn_pool.tile([128, NPR, CH], F32R, tag="attn",
                                     name=f"attn{c}")
            for hg in range(2):
                heads = [4 * hg + i for i in range(4)]
                pv = {}
                for h in heads:
                    pv[h] = ps_pv.tile([HD + 1, CH], F32,
                                       tag=f"pv{h % 4}", name=f"pv_{c}_{h}")
                for k in range(K):
                    for pr in (2 * hg, 2 * hg + 1):
                        for sub in range(2):
                            h = 2 * pr + sub
                            st = ps_st.tile([128, CH], F32, tag="st")
                            nc.tensor.matmul(
                                st[:],
                                lhsT=kT_sb[64 * sub:64 * (sub + 1), pr,
                                           128 * k:128 * (k + 1)],
                                rhs=qTc[64 * sub:64 * (sub + 1), pr, :],
                                start=True, stop=True)
                            e = e_pool.tile([128, CH], F32R, tag="e")
                            nc.scalar.activation(
                                out=e[:], in_=st[:],
                                func=mybir.ActivationFunctionType.Exp,
                                scale=0.125)
                            d = k - 4 * c
                            if d >= 0:
                                # causal: keep E[p, n] where n >= p + 128*d
                                nc.gpsimd.affine_select(
                                    out=e[:], in_=e[:],
                                    compare_op=mybir.AluOpType.is_ge,
                                    fill=0.0,
                                    base=-128 * d,
                                    pattern=[[1, CH]],
                                    channel_multiplier=-1)
                            nc.tensor.matmul(
                                pv[h][:],
                                lhsT=vaug[:, k, h, :],
                                rhs=e[:],
                                start=(k == 0), stop=(k == K - 1))
                for h in heads:
                    pr, sub = h // 2, h % 2
                    recip = sm_pool.tile([1, CH], F32R, tag="recip")
                    with nc.allow_low_precision(
                            reason="f32r recip feeds f32r matmul broadcast"):
                        nc.vector.reciprocal(recip[:], pv[h][HD:HD + 1, :])
                    bc_ps = ps_st.tile([HD, CH], F32, tag="st")
                    nc.tensor.matmul(bc_ps[:], lhsT=ones_bc[:],
                                     rhs=recip[:], start=True, stop=True)
                    bc = bc_pool.tile([HD, CH], F32, tag="bc")
                    nc.vector.tensor_copy(bc[:], bc_ps[:])
                    nc.vector.tensor_mul(
                        attn_sb[64 * sub:64 * (sub + 1), pr, :],
                        pv[h][0:HD, :], bc[:])
            # ---- output projection for chunk q ----
            for it in range(D // 128):
                ps = ps_mm.tile([128, CH], F32, tag="mm")
                for jt in range(NPR):
                    nc.tensor.matmul(
                        ps[:],
                        lhsT=wo_sb[:, jt, 128 * it:128 * (it + 1)],
                        rhs=attn_sb[:, jt, :],
                        start=(jt == 0), stop=(jt == NPR - 1))
                stg = out_pool.tile([128, CH], F32, tag="stg")
                nc.vector.tensor_copy(stg[:], ps[:])
                nc.sync.dma_start(
                    out=outT[128 * it:128 * (it + 1), CH * c:CH * (c + 1)],
                    in_=stg[:])


_PROGRAM = None


def _get_program():
    global _PROGRAM
    if _PROGRAM is None:
        _PROGRAM = build_program()
    return _PROGRAM


def kernel(x, Wq, Wk, Wv, Wo, bo):
    x = np.asarray(x, np.float32)
    Wq = np.asarray(Wq, np.float32)
    Wk = np.asarray(Wk, np.float32)
    Wv = np.asarray(Wv, np.float32)
    Wo = np.asarray(Wo, np.float32)
    bo = np.asarray(bo, np.float32)

    nc = _get_program()
    in_maps = []
    for c in range(N_CORES):
        b, g = c // GROUPS, c % GROUPS
        rows = slice(DG * g, DG * (g + 1))
        in_maps.append({
            "xT": np.ascontiguousarray(x[b].T),
            "wqT": np.ascontiguousarray(Wq[rows, :].T),
            "wkT": np.ascontiguousarray(Wk[rows, :].T),
            "wvT": np.ascontiguousarray(Wv[rows, :].T),
            "woT": np.ascontiguousarray(Wo[:, rows].T),
        })
    res = run_bass_kernel_spmd(nc, in_maps, core_ids=list(range(N_CORES)))
    out = np.empty((B, T, D), np.float32)
    for b in range(B):
        acc = res.results[GROUPS * b]["outT"].astype(np.float32)
        for g in range(1, GROUPS):
            acc = acc + res.results[GROUPS * b + g]["outT"]
        out[b] = acc.T + bo
    return out


# revision 15
# speedup vs baseline: 1.1769x; 1.1675x over previous
"""Multi-head causal attention (B=4, T=2048, D=1024, 16 heads) on 8 TRN2 cores.

Sharding: core c handles batch b = c//2 and head-group g = c%2 (8 of the 16
heads, i.e. 512 of the 1024 qkv feature dims).  Each core computes its head
group's QKV projections, causal attention, and a partial output projection
(columns of Wo belonging to its heads).  The host sums the two partials per
batch and adds the bias.

Per-core kernel layout (all matmul operands stored as float32r — full-rate
PE with ~1e-4 relative error):
  stage 1: qT[dh, t], kT[dh, t] (transposed) and v[t, dh] from xT j-tiles.
           v is stored with an interleaved ones-column per head ("vaug") so
           the PV matmul accumulates the softmax denominator for free.
  stage 2: per (tq-chunk 512, head): S^T blocks = kT_blk.T @ qT_chunk
           ([tk=128, tq=512] in PSUM), exp on ACT (no max subtraction --
           scores are O(1) so exp is safe in fp32), causal mask on the 4
           diagonal k-tiles via gpsimd affine_select, PV matmul accumulates
           outT[65, 512] (row 64 = denominator).  Normalize via DVE
           reciprocal + PE ones-broadcast + DVE multiply -> attnT[j, t].
  stage 3: partialT[i, t] = WoT_blk.T @ attnT chunks.
"""

import numpy as np

import concourse.bass as bass
import concourse.mybir as mybir
import concourse.tile as tile
from concourse.bass_utils import run_bass_kernel_spmd

B, T, D = 4, 2048, 1024
N_HEAD, HD = 16, 64
N_CORES = 8
GROUPS = 2            # head groups (cores per batch)
HPC = N_HEAD // GROUPS  # heads per core = 8
DG = HPC * HD           # feature dims per core = 512
NJT = D // 128          # 8 j-tiles over the model dim
NPR = DG // 128         # 4 dh-tiles (head pairs) per core
NTT = T // 128          # 16 t-tiles
NCH = T // 512          # 4 t-chunks
CH = 512

F32 = mybir.dt.float32
F32R = mybir.dt.float32r


def _split_excess_waits(nc, max_waits=1):
    """This walrus build encodes at most one sync-wait per instruction;
    Tile emits several.  Hoist surplus waits onto standalone same-engine
    NoOps placed immediately before the instruction."""
    for f in nc.m.functions:
        for bb in f.blocks:
            new = []
            for inst in bb.instructions:
                si = inst.sync_info
                waits = list(si.on_wait) if si is not None and si.on_wait else []
                if len(waits) > max_waits:
                    surplus, keep = waits[:-max_waits], waits[-max_waits:]
                    for k, w in enumerate(surplus):
                        nop = mybir.InstNoOp(name=f"{inst.name}-wsplit{k}", ins=[], outs=[])
                        nop.engine = inst.engine
                        nop.sync_info = mybir.SyncInfo(on_wait=[w], on_update=[])
                        new.append(nop)
                    inst.sync_info = mybir.SyncInfo(
                        on_wait=keep,
                        on_update=list(si.on_update) if si.on_update else [])
                new.append(inst)
            bb.instructions = new


def build_program():
    nc = bass.Bass("TRN2", target_bir_lowering=False, debug=False,
                   num_devices=N_CORES)

    xT = nc.dram_tensor("xT", [D, T], F32R, kind="ExternalInput")
    wqT = nc.dram_tensor("wqT", [D, DG], F32R, kind="ExternalInput")
    wkT = nc.dram_tensor("wkT", [D, DG], F32R, kind="ExternalInput")
    wvT = nc.dram_tensor("wvT", [D, DG], F32R, kind="ExternalInput")
    woT = nc.dram_tensor("woT", [DG, D], F32R, kind="ExternalInput")
    outT = nc.dram_tensor("outT", [D, T], F32, kind="ExternalOutput")

    with tile.TileContext(nc) as tc:
        _build_body(nc, tc, xT, wqT, wkT, wvT, woT, outT)
    _split_excess_waits(nc)
    return nc


def _build_body(nc, tc, xT, wqT, wkT, wvT, woT, outT):
    from contextlib import ExitStack
    est = ExitStack()
    with est:
        persist = est.enter_context(tc.tile_pool(name="persist", bufs=1))
        e_pool = est.enter_context(tc.tile_pool(name="epool", bufs=4))
        attn_pool = est.enter_context(tc.tile_pool(name="attnpool", bufs=2))
        qt_pool = est.enter_context(tc.tile_pool(name="qtpool", bufs=2))
        out_pool = est.enter_context(tc.tile_pool(name="outpool", bufs=2))
        bc_pool = est.enter_context(tc.tile_pool(name="bcpool", bufs=2))
        sm_pool = est.enter_context(tc.tile_pool(name="smpool", bufs=2))
        wpool = est.enter_context(tc.tile_pool(name="wqkv", bufs=1))
        wopool = est.enter_context(tc.tile_pool(name="wopool", bufs=1))
        xpool = est.enter_context(tc.tile_pool(name="xhpool", bufs=1))
        ps_mm = est.enter_context(tc.tile_pool(name="ps_mm", bufs=1, space="PSUM"))
        ps_st = est.enter_context(tc.tile_pool(name="ps_st", bufs=3, space="PSUM"))
        ps_pv = est.enter_context(tc.tile_pool(name="ps_pv", bufs=1, space="PSUM"))
        # psum budget: mm 1 + st 3 + pv 4 = 8 banks

        # persistent tensors
        kT_sb = persist.tile([128, NPR, T], F32R)     # [dh%128, dh-tile, t]
        vaug = persist.tile([128, NTT, HPC, HD + 1], F32R)  # [t%128, t-tile, h, hd|1]
        # Memset can't write f32r on this toolchain; stage fp32 ones and
        # copy (the DVE copy performs the f32r rounding).
        ones_f32 = persist.tile([128, HD], F32)
        nc.vector.memset(ones_f32[:], 1.0)
        ones_bc = persist.tile([1, HD], F32R)
        nc.vector.tensor_copy(ones_bc[:], ones_f32[0:1, :])
        for tt in range(NTT):
            nc.vector.tensor_copy(vaug[:, tt, :, HD], ones_f32[:, 0:HPC])

        wq_sb = wpool.tile([128, NJT, DG], F32R)
        wk_sb = wpool.tile([128, NJT, DG], F32R)
        wv_sb = wpool.tile([128, NJT, DG], F32R)
        wo_sb = wopool.tile([128, NPR, D], F32R)
        xh = {}

        def dma_xh(q):
            xh[q] = xpool.tile([128, NJT, CH], F32R, tag="xh", name=f"xh{q}")
            for jt in range(NJT):
                nc.sync.dma_start(out=xh[q][:, jt, :],
                                  in_=xT[128 * jt:128 * (jt + 1), CH * q:CH * (q + 1)])

        # First quarter of x and wq go first so the PE can start ASAP.
        dma_xh(0)
        for jt in range(NJT):
            nc.sync.dma_start(out=wq_sb[:, jt, :], in_=wqT[128 * jt:128 * (jt + 1), :])
        for jt in range(NJT):
            nc.sync.dma_start(out=wk_sb[:, jt, :], in_=wkT[128 * jt:128 * (jt + 1), :])
        for jt in range(NJT):
            nc.sync.dma_start(out=wv_sb[:, jt, :], in_=wvT[128 * jt:128 * (jt + 1), :])
        for jt in range(NPR):
            nc.sync.dma_start(out=wo_sb[:, jt, :], in_=woT[128 * jt:128 * (jt + 1), :])

        qTc_tiles = {}

        def emit_s1_group(q, kind, idx):
            """One stage-1 psum group (8 accumulating MMs + 1 copy) for
            t-quarter q.  kind: 'q'/'k' (idx = dh-tile) or 'v' (idx = t-tile
            within quarter)."""
            if kind in ("q", "k"):
                wsb = wq_sb if kind == "q" else wk_sb
                ps = ps_mm.tile([128, CH], F32, tag="mm", name=f"s1_{kind}{q}_{idx}")
                for jt in range(NJT):
                    nc.tensor.matmul(
                        ps[:],
                        lhsT=wsb[:, jt, 128 * idx:128 * (idx + 1)],
                        rhs=xh[q][:, jt, :],
                        start=(jt == 0), stop=(jt == NJT - 1))
                if kind == "q":
                    nc.vector.tensor_copy(qTc_tiles[q][:, idx, :], ps[:])
                else:
                    nc.vector.tensor_copy(kT_sb[:, idx, CH * q:CH * (q + 1)], ps[:])
            else:
                ps = ps_mm.tile([128, DG], F32, tag="mm", name=f"s1_v{q}_{idx}")
                for jt in range(NJT):
                    nc.tensor.matmul(
                        ps[:],
                        lhsT=xh[q][:, jt, 128 * idx:128 * (idx + 1)],
                        rhs=wv_sb[:, jt, :],
                        start=(jt == 0), stop=(jt == NJT - 1))
                nc.vector.tensor_copy(
                    vaug[:, 4 * q + idx, :, 0:HD],
                    ps[:].rearrange("p (h d) -> p h d", h=HPC))

        def s1_fillers(q):
            qTc_tiles[q] = qt_pool.tile([128, NPR, CH], F32R, tag="qT",
                                        name=f"qT{q}")
            fs = [lambda q=q, i=i: emit_s1_group(q, "q", i) for i in range(NPR)]
            fs += [lambda q=q, i=i: emit_s1_group(q, "k", i) for i in range(NPR)]
            fs += [lambda q=q, i=i: emit_s1_group(q, "v", i) for i in range(4)]
            return fs

        def emit_oproj_group(c, it, psum_tag="mm"):
            attn_sb = attn_tiles[c]
            ps = ps_mm.tile([128, CH], F32, tag=psum_tag, name=f"op_{c}_{it}") \
                if psum_tag == "mm" else \
                ps_st.tile([128, CH], F32, tag="st", name=f"op_{c}_{it}")
            for jt in range(NPR):
                nc.tensor.matmul(
                    ps[:],
                    lhsT=wo_sb[:, jt, 128 * it:128 * (it + 1)],
                    rhs=attn_sb[:, jt, :],
                    start=(jt == 0), stop=(jt == NPR - 1))
            stg = out_pool.tile([128, CH], F32, tag="stg")
            nc.vector.tensor_copy(stg[:], ps[:])
            nc.sync.dma_start(
                out=outT[128 * it:128 * (it + 1), CH * c:CH * (c + 1)],
                in_=stg[:])

        attn_tiles = {}

        def emit_attention_chunk(c, fillers):
            """Attention for tq chunk c, with filler groups interleaved to
            keep the PE fed while ACT works through the exps."""
            K = 4 * (c + 1)
            qTc = qTc_tiles[c]
            attn_tiles[c] = attn_pool.tile([128, NPR, CH], F32R, tag="attn",
                                           name=f"attn{c}")
            attn_sb = attn_tiles[c]
            n_units = K * 4  # (k, pr) pairs
            credit, unit = 0.0, 0
            rate = len(fillers) / max(1, n_units)
            fq = list(fillers)
            for hg in range(2):
                heads = [4 * hg + i for i in range(4)]
                pv = {}
                for h in heads:
                    pv[h] = ps_pv.tile([HD + 1, CH], F32,
                                       tag=f"pv{h % 4}", name=f"pv_{c}_{h}")
                for k in range(K):
                    for pr in (2 * hg, 2 * hg + 1):
                        d = k - 4 * c
                        o = 0 if d < 1 else min(128 * d, CH - 256)
                        for sub in range(2):
                            h = 2 * pr + sub
                            st = ps_st.tile([128, CH - o], F32, tag="st",
                                            name=f"st_{c}_{h}_{k}")
                            nc.tensor.matmul(
                                st[:],
                                lhsT=kT_sb[64 * sub:64 * (sub + 1), pr,
                                           128 * k:128 * (k + 1)],
                                rhs=qTc[64 * sub:64 * (sub + 1), pr, o:],
                                start=True, stop=True)
                            e = e_pool.tile([128, CH], F32R, tag="e")
                            nc.scalar.activation(
                                out=e[:, o:], in_=st[:],
                                func=mybir.ActivationFunctionType.Exp,
                                scale=0.125)
                            if d >= 0:
                                # causal: keep E[p, n] where n >= p + 128*d
                                # (also fills the unwritten [0:o) with 0)
                                nc.gpsimd.affine_select(
                                    out=e[:], in_=e[:],
                                    compare_op=mybir.AluOpType.is_ge,
                                    fill=0.0,
                                    base=-128 * d,
                                    pattern=[[1, CH]],
                                    channel_multiplier=-1)
                            nc.tensor.matmul(
                                pv[h][:],
                                lhsT=vaug[:, k, h, :],
                                rhs=e[:],
                                start=(k == 0), stop=(k == K - 1))
                        unit += 1
                        credit += rate
                        while credit >= 1.0 and fq:
                            fq.pop(0)()
                            credit -= 1.0
                for h in heads:
                    pr, sub = h // 2, h % 2
                    recip = sm_pool.tile([1, CH], F32R, tag="recip")
                    with nc.allow_low_precision(
                            reason="f32r recip feeds f32r matmul broadcast"):
                        nc.vector.reciprocal(recip[:], pv[h][HD:HD + 1, :])
                    bc_ps = ps_st.tile([HD, CH], F32, tag="st",
                                       name=f"bc_{c}_{h}")
                    nc.tensor.matmul(bc_ps[:], lhsT=ones_bc[:],
                                     rhs=recip[:], start=True, stop=True)
                    bc = bc_pool.tile([HD, CH], F32, tag="bc")
                    nc.vector.tensor_copy(bc[:], bc_ps[:])
                    nc.vector.tensor_mul(
                        attn_sb[64 * sub:64 * (sub + 1), pr, :],
                        pv[h][0:HD, :], bc[:])
            for f in fq:
                f()

        # quarter 0 projections run standalone (nothing to overlap with yet)
        for f in s1_fillers(0):
            f()
        for c in range(NCH):
            fillers = []
            if c >= 1:
                fillers += [lambda c=c, it=it: emit_oproj_group(c - 1, it)
                            for it in range(D // 128)]
            if c + 1 < NCH:
                dma_xh(c + 1)
                fillers += s1_fillers(c + 1)
            emit_attention_chunk(c, fillers)
        # tail output projection (use the now-idle st psum slots)
        for it in range(D // 128):
            emit_oproj_group(NCH - 1, it, psum_tag="st")


_PROGRAM = None


def _get_program():
    global _PROGRAM
    if _PROGRAM is None:
        _PROGRAM = build_program()
    return _PROGRAM


def kernel(x, Wq, Wk, Wv, Wo, bo):
    x = np.asarray(x, np.float32)
    Wq = np.asarray(Wq, np.float32)
    Wk = np.asarray(Wk, np.float32)
    Wv = np.asarray(Wv, np.float32)
    Wo = np.asarray(Wo, np.float32)
    bo = np.asarray(bo, np.float32)

    nc = _get_program()
    in_maps = []
    for c in range(N_CORES):
        b, g = c // GROUPS, c % GROUPS
        rows = slice(DG * g, DG * (g + 1))
        in_maps.append({
            "xT": np.ascontiguousarray(x[b].T),
            "wqT": np.ascontiguousarray(Wq[rows, :].T),
            "wkT": np.ascontiguousarray(Wk[rows, :].T),
            "wvT": np.ascontiguousarray(Wv[rows, :].T),
            "woT": np.ascontiguousarray(Wo[:, rows].T),
        })
    res = run_bass_kernel_spmd(nc, in_maps, core_ids=list(range(N_CORES)))
    out = np.empty((B, T, D), np.float32)
    for b in range(B):
        acc = res.results[GROUPS * b]["outT"].astype(np.float32)
        for g in range(1, GROUPS):
            acc = acc + res.results[GROUPS * b + g]["outT"]
        out[b] = acc.T + bo
    return out


# revision 16
# speedup vs baseline: 1.1810x; 1.0035x over previous
"""Multi-head causal attention (B=4, T=2048, D=1024, 16 heads) on 8 TRN2 cores.

Sharding: core c handles batch b = c//2 and head-group g = c%2 (8 of the 16
heads, i.e. 512 of the 1024 qkv feature dims).  Each core computes its head
group's QKV projections, causal attention, and a partial output projection
(columns of Wo belonging to its heads).  The host sums the two partials per
batch and adds the bias.

Per-core kernel layout (all matmul operands stored as float32r — full-rate
PE with ~1e-4 relative error):
  stage 1: qT[dh, t], kT[dh, t] (transposed) and v[t, dh] from xT j-tiles.
           v is stored with an interleaved ones-column per head ("vaug") so
           the PV matmul accumulates the softmax denominator for free.
  stage 2: per (tq-chunk 512, head): S^T blocks = kT_blk.T @ qT_chunk
           ([tk=128, tq=512] in PSUM), exp on ACT (no max subtraction --
           scores are O(1) so exp is safe in fp32), causal mask on the 4
           diagonal k-tiles via gpsimd affine_select, PV matmul accumulates
           outT[65, 512] (row 64 = denominator).  Normalize via DVE
           reciprocal + PE ones-broadcast + DVE multiply -> attnT[j, t].
  stage 3: partialT[i, t] = WoT_blk.T @ attnT chunks.
"""

import numpy as np

import concourse.bass as bass
import concourse.mybir as mybir
import concourse.tile as tile
from concourse.bass_utils import run_bass_kernel_spmd

B, T, D = 4, 2048, 1024
N_HEAD, HD = 16, 64
N_CORES = 8
GROUPS = 2            # head groups (cores per batch)
HPC = N_HEAD // GROUPS  # heads per core = 8
DG = HPC * HD           # feature dims per core = 512
NJT = D // 128          # 8 j-tiles over the model dim
NPR = DG // 128         # 4 dh-tiles (head pairs) per core
NTT = T // 128          # 16 t-tiles
NCH = T // 512          # 4 t-chunks
CH = 512

F32 = mybir.dt.float32
F32R = mybir.dt.float32r


def _split_excess_waits(nc, max_waits=1):
    """This walrus build encodes at most one sync-wait per instruction;
    Tile emits several.  Hoist surplus waits onto standalone same-engine
    NoOps placed immediately before the instruction."""
    for f in nc.m.functions:
        for bb in f.blocks:
            new = []
            for inst in bb.instructions:
                si = inst.sync_info
                waits = list(si.on_wait) if si is not None and si.on_wait else []
                if len(waits) > max_waits:
                    surplus, keep = waits[:-max_waits], waits[-max_waits:]
                    for k, w in enumerate(surplus):
                        nop = mybir.InstNoOp(name=f"{inst.name}-wsplit{k}", ins=[], outs=[])
                        nop.engine = inst.engine
                        nop.sync_info = mybir.SyncInfo(on_wait=[w], on_update=[])
                        new.append(nop)
                    inst.sync_info = mybir.SyncInfo(
                        on_wait=keep,
                        on_update=list(si.on_update) if si.on_update else [])
                new.append(inst)
            bb.instructions = new


def build_program():
    nc = bass.Bass("TRN2", target_bir_lowering=False, debug=False,
                   num_devices=N_CORES)

    xT = nc.dram_tensor("xT", [D, T], F32R, kind="ExternalInput")
    wqT = nc.dram_tensor("wqT", [D, DG], F32R, kind="ExternalInput")
    wkT = nc.dram_tensor("wkT", [D, DG], F32R, kind="ExternalInput")
    wvT = nc.dram_tensor("wvT", [D, DG], F32R, kind="ExternalInput")
    woT = nc.dram_tensor("woT", [DG, D], F32R, kind="ExternalInput")
    outT = nc.dram_tensor("outT", [D, T], F32, kind="ExternalOutput")

    with tile.TileContext(nc) as tc:
        _build_body(nc, tc, xT, wqT, wkT, wvT, woT, outT)
    _split_excess_waits(nc)
    return nc


def _build_body(nc, tc, xT, wqT, wkT, wvT, woT, outT):
    from contextlib import ExitStack
    est = ExitStack()
    with est:
        persist = est.enter_context(tc.tile_pool(name="persist", bufs=1))
        e_pool = est.enter_context(tc.tile_pool(name="epool", bufs=4))
        attn_pool = est.enter_context(tc.tile_pool(name="attnpool", bufs=3))
        qt_pool = est.enter_context(tc.tile_pool(name="qtpool", bufs=2))
        out_pool = est.enter_context(tc.tile_pool(name="outpool", bufs=2))
        bc_pool = est.enter_context(tc.tile_pool(name="bcpool", bufs=2))
        sm_pool = est.enter_context(tc.tile_pool(name="smpool", bufs=2))
        wpool = est.enter_context(tc.tile_pool(name="wqkv", bufs=1))
        wopool = est.enter_context(tc.tile_pool(name="wopool", bufs=1))
        xpool = est.enter_context(tc.tile_pool(name="xhpool", bufs=1))
        ps_mm = est.enter_context(tc.tile_pool(name="ps_mm", bufs=1, space="PSUM"))
        ps_st = est.enter_context(tc.tile_pool(name="ps_st", bufs=3, space="PSUM"))
        ps_pv = est.enter_context(tc.tile_pool(name="ps_pv", bufs=1, space="PSUM"))
        # psum budget: mm 1 + st 3 + pv 4 = 8 banks

        # persistent tensors
        kT_sb = persist.tile([128, NPR, T], F32R)     # [dh%128, dh-tile, t]
        vaug = persist.tile([128, NTT, HPC, HD + 1], F32R)  # [t%128, t-tile, h, hd|1]
        # Memset can't write f32r on this toolchain; stage fp32 ones and
        # copy (the DVE copy performs the f32r rounding).
        ones_f32 = persist.tile([128, HD], F32)
        nc.vector.memset(ones_f32[:], 1.0)
        ones_bc = persist.tile([1, HD], F32R)
        nc.vector.tensor_copy(ones_bc[:], ones_f32[0:1, :])
        for tt in range(NTT):
            nc.vector.tensor_copy(vaug[:, tt, :, HD], ones_f32[:, 0:HPC])

        wq_sb = wpool.tile([128, NJT, DG], F32R)
        wk_sb = wpool.tile([128, NJT, DG], F32R)
        wv_sb = wpool.tile([128, NJT, DG], F32R)
        wo_sb = wopool.tile([128, NPR, D], F32R)
        xh = {}

        def dma_xh(q):
            xh[q] = xpool.tile([128, NJT, CH], F32R, tag="xh", name=f"xh{q}")
            for jt in range(NJT):
                nc.sync.dma_start(out=xh[q][:, jt, :],
                                  in_=xT[128 * jt:128 * (jt + 1), CH * q:CH * (q + 1)])

        # First quarter of x and wq go first, interleaved per j-tile so the
        # first q-projection matmul can start after ~2 transfers.
        xh[0] = xpool.tile([128, NJT, CH], F32R, tag="xh", name="xh0")
        for jt in range(NJT):
            nc.sync.dma_start(out=xh[0][:, jt, :],
                              in_=xT[128 * jt:128 * (jt + 1), 0:CH])
            nc.sync.dma_start(out=wq_sb[:, jt, :], in_=wqT[128 * jt:128 * (jt + 1), :])
        for jt in range(NJT):
            nc.sync.dma_start(out=wk_sb[:, jt, :], in_=wkT[128 * jt:128 * (jt + 1), :])
        for jt in range(NJT):
            nc.sync.dma_start(out=wv_sb[:, jt, :], in_=wvT[128 * jt:128 * (jt + 1), :])
        for jt in range(NPR):
            nc.sync.dma_start(out=wo_sb[:, jt, :], in_=woT[128 * jt:128 * (jt + 1), :])

        qTc_tiles = {}

        def emit_s1_group(q, kind, idx):
            """One stage-1 psum group (8 accumulating MMs + 1 copy) for
            t-quarter q.  kind: 'q'/'k' (idx = dh-tile) or 'v' (idx = t-tile
            within quarter)."""
            if kind in ("q", "k"):
                wsb = wq_sb if kind == "q" else wk_sb
                ps = ps_mm.tile([128, CH], F32, tag="mm", name=f"s1_{kind}{q}_{idx}")
                for jt in range(NJT):
                    nc.tensor.matmul(
                        ps[:],
                        lhsT=wsb[:, jt, 128 * idx:128 * (idx + 1)],
                        rhs=xh[q][:, jt, :],
                        start=(jt == 0), stop=(jt == NJT - 1))
                if kind == "q":
                    nc.vector.tensor_copy(qTc_tiles[q][:, idx, :], ps[:])
                else:
                    nc.vector.tensor_copy(kT_sb[:, idx, CH * q:CH * (q + 1)], ps[:])
            else:
                ps = ps_mm.tile([128, DG], F32, tag="mm", name=f"s1_v{q}_{idx}")
                for jt in range(NJT):
                    nc.tensor.matmul(
                        ps[:],
                        lhsT=xh[q][:, jt, 128 * idx:128 * (idx + 1)],
                        rhs=wv_sb[:, jt, :],
                        start=(jt == 0), stop=(jt == NJT - 1))
                nc.vector.tensor_copy(
                    vaug[:, 4 * q + idx, :, 0:HD],
                    ps[:].rearrange("p (h d) -> p h d", h=HPC))

        def s1_fillers(q):
            qTc_tiles[q] = qt_pool.tile([128, NPR, CH], F32R, tag="qT",
                                        name=f"qT{q}")
            fs = [lambda q=q, i=i: emit_s1_group(q, "q", i) for i in range(NPR)]
            fs += [lambda q=q, i=i: emit_s1_group(q, "k", i) for i in range(NPR)]
            fs += [lambda q=q, i=i: emit_s1_group(q, "v", i) for i in range(4)]
            return fs

        def emit_oproj_group(c, it, psum_tag="mm"):
            attn_sb = attn_tiles[c]
            ps = ps_mm.tile([128, CH], F32, tag=psum_tag, name=f"op_{c}_{it}") \
                if psum_tag == "mm" else \
                ps_st.tile([128, CH], F32, tag="st", name=f"op_{c}_{it}")
            for jt in range(NPR):
                nc.tensor.matmul(
                    ps[:],
                    lhsT=wo_sb[:, jt, 128 * it:128 * (it + 1)],
                    rhs=attn_sb[:, jt, :],
                    start=(jt == 0), stop=(jt == NPR - 1))
            stg = out_pool.tile([128, CH], F32, tag="stg")
            nc.vector.tensor_copy(stg[:], ps[:])
            nc.sync.dma_start(
                out=outT[128 * it:128 * (it + 1), CH * c:CH * (c + 1)],
                in_=stg[:])

        attn_tiles = {}

        def emit_attention_chunk(c, fillers):
            """Attention for tq chunk c, with filler groups interleaved to
            keep the PE fed while ACT works through the exps."""
            K = 4 * (c + 1)
            qTc = qTc_tiles[c]
            attn_tiles[c] = attn_pool.tile([128, NPR, CH], F32R, tag="attn",
                                           name=f"attn{c}")
            attn_sb = attn_tiles[c]
            n_units = K * 4  # (k, pr) pairs
            credit, unit = 0.0, 0
            rate = len(fillers) / max(1, n_units)
            fq = list(fillers)
            for hg in range(2):
                heads = [4 * hg + i for i in range(4)]
                pv = {}
                for h in heads:
                    pv[h] = ps_pv.tile([HD + 1, CH], F32,
                                       tag=f"pv{h % 4}", name=f"pv_{c}_{h}")
                for k in range(K):
                    for pr in (2 * hg, 2 * hg + 1):
                        d = k - 4 * c
                        o = 0 if d < 1 else min(128 * d, CH - 256)
                        for sub in range(2):
                            h = 2 * pr + sub
                            st = ps_st.tile([128, CH - o], F32, tag="st",
                                            name=f"st_{c}_{h}_{k}")
                            nc.tensor.matmul(
                                st[:],
                                lhsT=kT_sb[64 * sub:64 * (sub + 1), pr,
                                           128 * k:128 * (k + 1)],
                                rhs=qTc[64 * sub:64 * (sub + 1), pr, o:],
                                start=True, stop=True)
                            e = e_pool.tile([128, CH], F32R, tag="e")
                            nc.scalar.activation(
                                out=e[:, o:], in_=st[:],
                                func=mybir.ActivationFunctionType.Exp,
                                scale=0.125)
                            if d >= 0:
                                # causal: keep E[p, n] where n >= p + 128*d
                                # (also fills the unwritten [0:o) with 0)
                                nc.gpsimd.affine_select(
                                    out=e[:], in_=e[:],
                                    compare_op=mybir.AluOpType.is_ge,
                                    fill=0.0,
                                    base=-128 * d,
                                    pattern=[[1, CH]],
                                    channel_multiplier=-1)
                            nc.tensor.matmul(
                                pv[h][:],
                                lhsT=vaug[:, k, h, :],
                                rhs=e[:],
                                start=(k == 0), stop=(k == K - 1))
                        unit += 1
                        credit += rate
                        while credit >= 1.0 and fq:
                            fq.pop(0)()
                            credit -= 1.0
                for h in heads:
                    pr, sub = h // 2, h % 2
                    recip = sm_pool.tile([1, CH], F32R, tag="recip")
                    with nc.allow_low_precision(
                            reason="f32r recip feeds f32r matmul broadcast"):
                        nc.vector.reciprocal(recip[:], pv[h][HD:HD + 1, :])
                    bc_ps = ps_st.tile([HD, CH], F32, tag="st",
                                       name=f"bc_{c}_{h}")
                    nc.tensor.matmul(bc_ps[:], lhsT=ones_bc[:],
                                     rhs=recip[:], start=True, stop=True)
                    bc = bc_pool.tile([HD, CH], F32, tag="bc")
                    nc.vector.tensor_copy(bc[:], bc_ps[:])
                    nc.vector.tensor_mul(
                        attn_sb[64 * sub:64 * (sub + 1), pr, :],
                        pv[h][0:HD, :], bc[:])
            for f in fq:
                f()

        # quarter 0 projections run standalone (nothing to overlap with yet)
        for f in s1_fillers(0):
            f()
        # outproj chunks are deferred as late as dependencies allow, to feed
        # the PE during the ACT-heavy late attention chunks:
        #   chunk2 <- opj0 + s1q3; chunk3 <- opj1 + opj2; tail <- opj3
        for c in range(NCH):
            fillers = []
            if c == 2:
                fillers += [lambda it=it: emit_oproj_group(0, it)
                            for it in range(D // 128)]
            elif c == 3:
                fillers += [lambda it=it: emit_oproj_group(1, it)
                            for it in range(D // 128)]
                fillers += [lambda it=it: emit_oproj_group(2, it)
                            for it in range(D // 128)]
            if c + 1 < NCH:
                dma_xh(c + 1)
                fillers += s1_fillers(c + 1)
            emit_attention_chunk(c, fillers)
        # tail output projection (use the now-idle st psum slots)
        for it in range(D // 128):
            emit_oproj_group(NCH - 1, it, psum_tag="st")


_PROGRAM = None


def _get_program():
    global _PROGRAM
    if _PROGRAM is None:
        _PROGRAM = build_program()
    return _PROGRAM


def kernel(x, Wq, Wk, Wv, Wo, bo):
    x = np.asarray(x, np.float32)
    Wq = np.asarray(Wq, np.float32)
    Wk = np.asarray(Wk, np.float32)
    Wv = np.asarray(Wv, np.float32)
    Wo = np.asarray(Wo, np.float32)
    bo = np.asarray(bo, np.float32)

    nc = _get_program()
    in_maps = []
    for c in range(N_CORES):
        b, g = c // GROUPS, c % GROUPS
        rows = slice(DG * g, DG * (g + 1))
        in_maps.append({
            "xT": np.ascontiguousarray(x[b].T),
            "wqT": np.ascontiguousarray(Wq[rows, :].T),
            "wkT": np.ascontiguousarray(Wk[rows, :].T),
            "wvT": np.ascontiguousarray(Wv[rows, :].T),
            "woT": np.ascontiguousarray(Wo[:, rows].T),
        })
    res = run_bass_kernel_spmd(nc, in_maps, core_ids=list(range(N_CORES)))
    out = np.empty((B, T, D), np.float32)
    for b in range(B):
        acc = res.results[GROUPS * b]["outT"].astype(np.float32)
        for g in range(1, GROUPS):
            acc = acc + res.results[GROUPS * b + g]["outT"]
        out[b] = acc.T + bo
    return out
